# revision 1
# baseline (speedup 1.0000x reference)
"""DiffLogic network TRN2 kernel: 3 logic layers [B=256, W=64000] + GroupSum.

Sharding: pure data-parallel over batch across 8 cores (B=32/core), no
inter-core communication. Per core: activations h stored in DRAM as
[64000, 128] bf16 rows (256B, 32 real batch cols). Gathers a=h[idx_a],
b=h[idx_b] via SWDGE dma_gather with signed int16 indices (base at row
32000 so idx in [-32000, 32000)). Gate = c0+c1*a+c2*b+c3*ab computed on
DVE with stride-0 coefficient broadcasts; coefs = softmax(w)@G computed
on device (ACT exp + DVE reduce). GroupSum via PE one-hot matmul.
"""
import numpy as np
import ml_dtypes

import concourse.bass as bass
import concourse.tile as tile
import concourse.bacc as bacc
import concourse.mybir as mybir
from concourse.bass_utils import run_bass_kernel_spmd
from concourse.library_config import mlp
from concourse._compat import cdiv

W = 64000
BATCH = 256
NCORES = 8
BC = BATCH // NCORES        # 32 batch rows per core
IN_DIM = 1024
K = 10
TAU = 30.0
NSLOT = W // 128            # 500
E = 128                     # bf16 elements per h row (256B); [:32] real
CHUNK_SLOTS = 64            # neurons per chunk = 64*128 = 8192
GPN = 1024                  # idxs per dma_gather instruction
H_BASE = 32000              # gather base row (signed int16 rebase)

GATE_COEF = np.array([
    [0., 0., 0., 0.], [0., 0., 0., 1.], [0., 1., 0., -1.], [0., 1., 0., 0.],
    [0., 0., 1., -1.], [0., 0., 1., 0.], [0., 1., 1., -2.], [0., 1., 1., -1.],
    [1., -1., -1., 1.], [1., -1., -1., 2.], [1., 0., -1., 0.], [1., 0., -1., 1.],
    [1., -1., 0., 0.], [1., -1., 0., 1.], [1., 0., 0., -1.], [1., 0., 0., 0.],
], dtype=np.float32)  # [16, 4]

BF16 = mybir.dt.bfloat16
F32 = mybir.dt.float32
I16 = mybir.dt.int16
IDX_COLS = W // 16  # wrapped idx tensor cols per list

_NC_CACHE = {}


def _chunks():
    """Yield (slot0, nslots) chunks over the 500 slots."""
    s = 0
    while s < NSLOT:
        n = min(CHUNK_SLOTS, NSLOT - s)
        yield s, n
        s += n


def _gathers(nslots):
    """Split a chunk of nslots*128 idxs into per-instruction counts."""
    n = nslots * 128
    out = []
    while n > 0:
        g = min(GPN, n)
        out.append(g)
        n -= g
    return out


def build_nc():
    if "nc" in _NC_CACHE:
        return _NC_CACHE["nc"]
    nc = bacc.Bacc("TRN2", target_bir_lowering=False, debug=False,
                   enable_asserts=False, num_devices=NCORES)

    xT = nc.dram_tensor("xT", [IN_DIM, E], BF16, kind="ExternalInput")
    wf = [nc.dram_tensor(f"wf{l}", [128, NSLOT, 16], BF16, kind="ExternalInput")
          for l in range(3)]
    ia = [nc.dram_tensor(f"ia{l}", [128, IDX_COLS], I16, kind="ExternalInput")
          for l in range(3)]
    ib = [nc.dram_tensor(f"ib{l}", [128, IDX_COLS], I16, kind="ExternalInput")
          for l in range(3)]
    g10 = nc.dram_tensor("g10", [128, NSLOT, K], BF16, kind="ExternalInput")
    gmat = nc.dram_tensor("gmat", [128, 5, 16], BF16, kind="ExternalInput")
    h_dram = [nc.dram_tensor(f"h{l}", [W, E], BF16, kind="Internal")
              for l in range(2)]
    out_dram = nc.dram_tensor("out", [K, BC], F32, kind="ExternalOutput")

    with tile.TileContext(nc) as tc:
        with (
            tc.tile_pool(name="persist", bufs=1) as persist,
            tc.tile_pool(name="coef", bufs=1) as coefp,
            tc.tile_pool(name="gath", bufs=2) as gath,
            tc.tile_pool(name="temps", bufs=2) as temps,
            tc.tile_pool(name="psum", bufs=1, space="PSUM") as psump,
        ):
            nc.gpsimd.load_library(mlp)

            # persistent loads
            gmat_sb = persist.tile([128, 5, 16], BF16, tag="gmat")
            nc.sync.dma_start(gmat_sb[:], gmat[:])
            g10_sb = persist.tile([128, NSLOT, K], BF16, tag="g10")
            nc.sync.dma_start(g10_sb[:], g10[:])

            psum_out = psump.tile([K, BC], F32, tag="acc")
            n_mm = NSLOT  # total groupsum matmuls
            mm_i = 0

            for l in range(3):
                ia_sb = persist.tile([128, IDX_COLS], I16, tag="ia", name="ia_sb")
                ib_sb = persist.tile([128, IDX_COLS], I16, tag="ib", name="ib_sb")
                nc.sync.dma_start(ia_sb[:], ia[l][:])
                nc.sync.dma_start(ib_sb[:], ib[l][:])

                # ---- coefficient prep: coef = softmax(wf) @ GATE_COEF ----
                HS = NSLOT // 2
                cj = [coefp.tile([128, NSLOT], BF16, tag=f"c{j}", name=f"cj{j}") for j in range(4)]
                for h in range(2):
                    hs = slice(h * HS, (h + 1) * HS)
                    wf_sb = coefp.tile([128, HS, 16], BF16, tag="wf", name="wf_sb")
                    nc.sync.dma_start(wf_sb[:], wf[l][:, hs, :])
                    e_sb = coefp.tile([128, HS, 16], BF16, tag="e", name="e_sb")
                    nc.scalar.activation(e_sb[:], wf_sb[:],
                                         mybir.ActivationFunctionType.Exp)
                    prod = coefp.tile([128, HS, 16], BF16, tag="prod", name="prod")
                    craw = [coefp.tile([128, HS], F32, tag=f"craw{j}", name=f"craw{j}")
                            for j in range(4)]
                    for j in range(4):
                        gj = gmat_sb[:, j, :].unsqueeze(1).to_broadcast([128, HS, 16])
                        nc.vector.tensor_mul(prod[:], e_sb[:], gj)
                        nc.vector.tensor_reduce(craw[j][:], prod[:],
                                                mybir.AxisListType.X,
                                                mybir.AluOpType.add)
                    ssum = coefp.tile([128, HS], F32, tag="ssum", name="ssum")
                    nc.vector.tensor_reduce(ssum[:], e_sb[:], mybir.AxisListType.X,
                                            mybir.AluOpType.add)
                    rinv = coefp.tile([128, HS], F32, tag="rinv", name="rinv")
                    nc.vector.reciprocal(out=rinv[:], in_=ssum[:])
                    for j in range(4):
                        nc.vector.tensor_mul(cj[j][:, hs], craw[j][:], rinv[:])

                # ---- gather + gate over chunks ----
                if l == 0:
                    src_ap = xT[:]
                else:
                    src_ap = h_dram[l - 1][H_BASE:W]

                for s0, ns in _chunks():
                    a_t = gath.tile([128, CHUNK_SLOTS, E], BF16, tag="a")
                    b_t = gath.tile([128, CHUNK_SLOTS, E], BF16, tag="b")
                    col = s0 * 8  # idx cols consumed so far (128/16 per slot)
                    slot = 0
                    for n in _gathers(ns):
                        ncols = n // 16
                        nslots_g = n // 128
                        nc.gpsimd.dma_gather(
                            a_t[:, slot:slot + nslots_g, :], src_ap,
                            ia_sb[:, col:col + ncols], n, n, E)
                        nc.gpsimd.dma_gather(
                            b_t[:, slot:slot + nslots_g, :], src_ap,
                            ib_sb[:, col:col + ncols], n, n, E)
                        col += ncols
                        slot += nslots_g

                    av = a_t[:, :ns, :32]
                    bv = b_t[:, :ns, :32]

                    def cbc(j):
                        return (cj[j][:, s0:s0 + ns].unsqueeze(-1)
                                .to_broadcast([128, ns, 32]))

                    t_t = temps.tile([128, CHUNK_SLOTS, 32], BF16, tag="t")
                    u_t = temps.tile([128, CHUNK_SLOTS, 32], BF16, tag="u")
                    v_t = temps.tile([128, CHUNK_SLOTS, 32], BF16, tag="v")
                    w_t = temps.tile([128, CHUNK_SLOTS, 32], BF16, tag="w")
                    nc.vector.tensor_mul(t_t[:, :ns, :], av, bv)
                    nc.vector.tensor_mul(u_t[:, :ns, :], t_t[:, :ns, :], cbc(3))
                    nc.vector.tensor_mul(v_t[:, :ns, :], av, cbc(1))
                    nc.vector.tensor_mul(w_t[:, :ns, :], bv, cbc(2))
                    nc.vector.tensor_add(u_t[:, :ns, :], u_t[:, :ns, :], v_t[:, :ns, :])
                    nc.vector.tensor_add(w_t[:, :ns, :], w_t[:, :ns, :], cbc(0))
                    nc.vector.tensor_add(t_t[:, :ns, :], u_t[:, :ns, :], w_t[:, :ns, :])

                    if l < 2:
                        # write rows (s0+c)*128+p of h_dram[l]
                        hap = h_dram[l].ap()
                        dst = hap[s0 * 128: s0 * 128 + ns * 128, :32]
                        dst = dst.rearrange("(c p) e -> p c e", p=128)
                        nc.gpsimd.dma_start(dst, t_t[:, :ns, :])
                    else:
                        for c in range(ns):
                            nc.tensor.matmul(
                                psum_out[:],
                                lhsT=g10_sb[:, s0 + c, :],
                                rhs=t_t[:, c, :],
                                start=(mm_i == 0),
                                stop=(mm_i == n_mm - 1),
                            )
                            mm_i += 1

            out_sb = persist.tile([K, BC], F32, tag="outsb")
            nc.scalar.mul(out_sb[:], psum_out[:], 1.0 / TAU)
            nc.sync.dma_start(out_dram[:], out_sb[:])

    nc.compile()
    _NC_CACHE["nc"] = nc
    return nc


def _wrap(idx):
    """Flat idx list [n] -> [128, n/16] int16 wrapped per 16 partitions,
    replicated to the 8 gpsimd cores."""
    n = idx.shape[0]
    arr = np.empty((128, n // 16), dtype=np.int16)
    blk = idx.reshape(n // 16, 16).T.astype(np.int16)
    for g in range(8):
        arr[g * 16:(g + 1) * 16, :] = blk
    return arr


def _fix_trailing(idx_a, idx_b):
    """Ensure the last idx of every GPN-sublist is >= 0 for both lists
    (SWDGE trims trailing negatives). Returns permuted lists + perm."""
    perm = np.arange(W)
    a = idx_a.copy()
    b = idx_b.copy()
    pos = 0
    for s0, ns in _chunks():
        for n in _gathers(ns):
            last = pos + n - 1
            if a[last] < 0 or b[last] < 0:
                ok = np.nonzero((a[pos:last] >= 0) & (b[pos:last] >= 0))[0]
                j = pos + int(ok[-1])
                for arr in (a, b, perm):
                    arr[last], arr[j] = arr[j], arr[last]
            pos += n
    return a, b, perm


def _fold(x):
    """[W, ...] -> [128, NSLOT, ...] with row n=(c*128+p) at [p, c]."""
    return np.ascontiguousarray(
        x.reshape(NSLOT, 128, *x.shape[1:]).transpose(1, 0, *range(2, x.ndim + 1)))


def kernel(x, w1, w2, w3, idx_a1, idx_b1, idx_a2, idx_b2, idx_a3, idx_b3):
    x = np.asarray(x, dtype=np.float32)
    ws = [np.asarray(w, dtype=np.float32) for w in (w1, w2, w3)]
    ias = [np.asarray(i).astype(np.int64) for i in (idx_a1, idx_a2, idx_a3)]
    ibs = [np.asarray(i).astype(np.int64) for i in (idx_b1, idx_b2, idx_b3)]

    nc = build_nc()

    # ---- host-side index translation / layout prep (shared across cores) ----
    # layer 0: sources are x columns (0..1023), no rebase needed
    a0, b0, perm0 = ias[0].copy(), ibs[0].copy(), np.arange(W)
    perms = [perm0]
    lists = [(a0, b0)]
    for l in (1, 2):
        inv_prev = np.empty(W, dtype=np.int64)
        inv_prev[perms[l - 1]] = np.arange(W)
        ra = inv_prev[ias[l]] - H_BASE
        rb = inv_prev[ibs[l]] - H_BASE
        ra2, rb2, perm = _fix_trailing(ra, rb)
        perms.append(perm)
        lists.append((ra2, rb2))

    shared = {}
    for l in range(3):
        a, b = lists[l]
        shared[f"ia{l}"] = _wrap(a)
        shared[f"ib{l}"] = _wrap(b)
        shared[f"wf{l}"] = _fold(ws[l][perms[l]]).astype(ml_dtypes.bfloat16)

    group = perms[2] // (W // K)          # group id of neuron at list pos j
    g10 = np.zeros((W, K), dtype=np.float32)
    g10[np.arange(W), group] = 1.0
    shared["g10"] = _fold(g10).astype(ml_dtypes.bfloat16)

    gm = np.zeros((5, 16), dtype=np.float32)
    gm[:4] = GATE_COEF.T
    gm[4] = 1.0
    shared["gmat"] = np.broadcast_to(gm, (128, 5, 16)).astype(ml_dtypes.bfloat16)

    in_maps = []
    for c in range(NCORES):
        xc = x[c * BC:(c + 1) * BC]               # [32, 1024]
        xt = np.zeros((IN_DIM, E), dtype=ml_dtypes.bfloat16)
        xt[:, :BC] = xc.T.astype(ml_dtypes.bfloat16)
        m = dict(shared)
        m["xT"] = xt
        in_maps.append(m)

    res = run_bass_kernel_spmd(nc, in_maps, core_ids=list(range(NCORES)))

    out = np.empty((BATCH, K), dtype=np.float32)
    for c in range(NCORES):
        out[c * BC:(c + 1) * BC] = res.results[c]["out"].T
    return out



# revision 15
# speedup vs baseline: 1.4775x; 1.4775x over previous
"""DiffLogic network TRN2 kernel: 3 logic layers [B=256, W=64000] + GroupSum.

Sharding: pure data-parallel over batch across 8 cores (B=32/core), no
inter-core communication.

Layer 1: inputs host-gathered (x[:, idx] is pure indexing), no device
gather. Layer 2: SWDGE dma_gather of 256B rows from h1. Layer 3: NO
gather at all — layer 2's consumer stream is ordered as a path cover of
layer 3's access graph, so h2 (64B rows) already contains every layer-3
input pair at adjacent rows; layer 3 just bulk-loads h2 sequentially and
evaluates gates on the 7 adjacent-row offsets of each 8-row block.

Softmax coefs: exp (ACT) + shared-subexpression add/sub tree over the 16
gate planes (all bf16 TensorTensor in 2x mode) instead of TensorReduce.
Gate coefs stored duplicated-in-pairs ([.., 2]) so the batch-broadcast
AP keeps an innermost stride-1 pair and every gate op runs in 2x mode.
GroupSum via PE one-hot matmul accumulation in PSUM.
"""
import numpy as np
import ml_dtypes

import concourse.bass as bass
import concourse.tile as tile
import concourse.bacc as bacc
import concourse.mybir as mybir
from concourse.bass_utils import run_bass_kernel_spmd
from concourse.library_config import mlp

W = 64000
BATCH = 256
NCORES = 8
BC = BATCH // NCORES        # 32 batch rows per core
IN_DIM = 1024
K = 10
TAU = 30.0
NSLOT = W // 128            # 500
E = 128                     # bf16 elements per h1 row (256B); [:32] real
CHUNK_SLOTS = 48
GPN = 1024                  # idxs per dma_gather instruction
H_BASE = 32000              # gather base row (signed int16 rebase)
CB3 = 16                    # layer-3 block-columns per chunk

# a<->b swap permutation of the 16 gates: gate g(b,a) == gate SWP[g](a,b)
SWP = np.array([0, 1, 4, 5, 2, 3, 6, 7, 8, 9, 12, 13, 10, 11, 14, 15])

BF16 = mybir.dt.bfloat16
F32 = mybir.dt.float32
I16 = mybir.dt.int16
ADD = mybir.AluOpType.add
SUB = mybir.AluOpType.subtract

_NC_CACHE = {}


def _chunks(nslot, step=CHUNK_SLOTS):
    s = 0
    while s < nslot:
        n = min(step, nslot - s)
        yield s, n
        s += n


def _gathers(nslots):
    n = nslots * 128
    out = []
    while n > 0:
        g = min(GPN, n)
        out.append(g)
        n -= g
    return out


def _coef_tree(nc, alloc, e_sb, cj, rinv_dst, nsl):
    """craw0..3 + ssum from the 16 exp gate planes via shared add/sub
    subexpressions; normalized into duplicated-pair coef tiles."""

    def pl(g):
        return e_sb[:, g, :]

    tmp = {}

    def mk(name, eng, op, x, y):
        slot = SLOT[name]
        dst = alloc(ALIAS.get(slot, slot))
        eng.tensor_tensor(out=dst[:, :nsl], in0=x, in1=y, op=op)
        tmp[name] = dst
        return dst[:, :nsl]

    def t(name):
        return tmp[name][:, :nsl]

    # names mapped to shared buffer slots (liveness-based reuse)
    # an op's out slot must never alias one of its own input slots
    SLOT = {"s01": "A", "s23": "B", "s45": "C", "s67": "D", "s89": "E",
            "sAB": "F", "sCD": "G", "sEF": "H",
            "q0": "L", "q1": "M", "q2": "N", "q3": "O",
            "c0": "I", "h01": "J", "ssum": "K",
            "c1a": "A2", "c1b": "C2", "c1": "F2", "c2": "D2",
            "t1": "P1", "t2": "P2", "t3": "P3", "t4": "P4", "t5": "P5",
            "t5d": "P6", "u1": "P7", "u2": "P1b", "u3": "P2b", "c3": "P3b"}
    ALIAS = {"A2": "A", "C2": "C", "F2": "F", "D2": "D",
             "P1b": "P1", "P2b": "P2", "P3b": "P3"}

    dve, po = nc.vector, nc.gpsimd
    for i, nm in enumerate(["s01", "s23", "s45", "s67", "s89", "sAB", "sCD", "sEF"]):
        mk(nm, dve, ADD, pl(2 * i), pl(2 * i + 1))
    mk("q0", dve, ADD, t("s01"), t("s23"))
    mk("q1", dve, ADD, t("s45"), t("s67"))
    mk("q2", dve, ADD, t("s89"), t("sAB"))
    mk("q3", dve, ADD, t("sCD"), t("sEF"))
    mk("c0", dve, ADD, t("q2"), t("q3"))
    mk("h01", dve, ADD, t("q0"), t("q1"))
    mk("ssum", dve, ADD, t("h01"), t("c0"))
    mk("c1a", dve, ADD, t("s23"), t("s67"))
    mk("c1b", dve, ADD, t("s89"), t("sCD"))
    mk("c1", dve, SUB, t("c1a"), t("c1b"))
    mk("c2", dve, SUB, t("q1"), t("q2"))
    # c3 = e1-e2-e4-2e6-e7+e8+2e9+e11+e13-e14 (Pool, offloads DVE)
    mk("t1", po, SUB, pl(1), pl(2))
    mk("t2", po, SUB, pl(8), pl(4))
    mk("t3", po, SUB, pl(11), pl(7))
    mk("t4", po, SUB, pl(13), pl(14))
    mk("t5", po, SUB, pl(9), pl(6))
    mk("t5d", po, ADD, t("t5"), t("t5"))
    mk("u1", po, ADD, t("t1"), t("t2"))
    mk("u2", po, ADD, t("t3"), t("t4"))
    mk("u3", po, ADD, t("u1"), t("u2"))
    mk("c3", po, ADD, t("u3"), t("t5d"))

    nc.vector.reciprocal(out=rinv_dst, in_=t("ssum"))
    for j, nm in enumerate(["c0", "c1", "c2", "c3"]):
        eng = dve if j < 2 else po
        for kdup in range(2):
            eng.tensor_tensor(out=cj[j][:, :, kdup], in0=t(nm),
                              in1=rinv_dst, op=mybir.AluOpType.mult)


def _gate(nc, av, bv, cbc, u_t, v_t, ns):
    """h = (c3*b + c1)*a + (c2*b + c0), written into u_t[:, :ns].
    av/bv: [128, ns, 32] APs; cbc(j): coef broadcast [128, ns, 16, 2]."""
    av4 = av.rearrange("p n (a b) -> p n a b", b=2)
    bv4 = bv.rearrange("p n (a b) -> p n a b", b=2)
    u4 = u_t[:, :ns, :, :]
    v4 = v_t[:, :ns, :, :]
    nc.vector.tensor_mul(u4, bv4, cbc(3))
    nc.vector.tensor_add(u4, u4, cbc(1))
    nc.vector.tensor_mul(u4, u4, av4)
    nc.vector.tensor_mul(v4, bv4, cbc(2))
    nc.vector.tensor_add(v4, v4, cbc(0))
    nc.vector.tensor_add(u4, u4, v4)
    return u_t[:, :ns, :, :].rearrange("p n a b -> p n (a b)")


def build_nc(nbins=None):
    if nbins is None:
        nbins = _NC_CACHE["last_nbins"]
    _NC_CACHE["last_nbins"] = nbins
    key = ("nc", nbins)
    if key in _NC_CACHE:
        return _NC_CACHE[key]
    n2rows = nbins * 8
    nslot2 = n2rows // 128
    nc3 = nbins // 128          # block-columns for layer 3
    nslot3 = nc3 * 7            # coef slots for layer 3 (c, j)-flattened

    nc = bacc.Bacc("TRN2", target_bir_lowering=False, debug=False,
                   enable_asserts=False, num_devices=NCORES)

    a1_d = nc.dram_tensor("a1", [128, NSLOT, 32], BF16, kind="ExternalInput")
    b1_d = nc.dram_tensor("b1", [128, NSLOT, 32], BF16, kind="ExternalInput")
    nslots = [NSLOT, nslot2, nslot3]
    wf = [nc.dram_tensor(f"wf{l}", [128, 16, nslots[l]], BF16,
                         kind="ExternalInput") for l in range(3)]
    ia2 = nc.dram_tensor("ia2", [128, n2rows // 16], I16, kind="ExternalInput")
    ib2 = nc.dram_tensor("ib2", [128, n2rows // 16], I16, kind="ExternalInput")
    g10 = nc.dram_tensor("g10", [128, nc3, 7, K], BF16, kind="ExternalInput")
    h1_dram = nc.dram_tensor("h1", [W, E], BF16, kind="Internal")
    h2_dram = nc.dram_tensor("h2", [n2rows, 32], BF16, kind="Internal")
    out_dram = nc.dram_tensor("out", [K, BC], F32, kind="ExternalOutput")

    with tile.TileContext(nc) as tc:
        with (
            tc.tile_pool(name="persist", bufs=1) as persist,
            tc.tile_pool(name="coef", bufs=1) as coefp,
            tc.tile_pool(name="gath", bufs=2) as gath,
            tc.tile_pool(name="temps", bufs=2) as temps,
            tc.tile_pool(name="psum", bufs=1, space="PSUM") as psump,
        ):
            nc.gpsimd.load_library(mlp)

            g10_sb = persist.tile([128, nc3, 7, K], BF16, tag="g10")
            nc.sync.dma_start(g10_sb[:], g10[:])

            psum_out = psump.tile([K, BC], F32, tag="acc")
            n_mm = nc3 * 7
            mm_i = 0

            maxsl = max(nslots)
            for l in range(3):
                nsl = nslots[l]
                wf_t = coefp.tile([128, 16, maxsl], BF16, tag="wf",
                                  name=f"wf_sb{l}")
                wf_sb = wf_t[:, :, :nsl]
                nc.sync.dma_start(wf_sb, wf[l][:])
                e_t = coefp.tile([128, 16, maxsl], BF16, tag="e",
                                 name=f"e_sb{l}")
                e_sb = e_t[:, :, :nsl]
                nc.scalar.activation(e_sb, wf_sb,
                                     mybir.ActivationFunctionType.Exp)
                cj_t = [coefp.tile([128, maxsl, 2], BF16, tag=f"c{j}",
                                   name=f"cj{l}{j}") for j in range(4)]
                cj = [t[:, :nsl, :] for t in cj_t]
                rinv_t = coefp.tile([128, maxsl], F32, tag="rinv",
                                    name=f"rinv{l}")
                rinv = rinv_t[:, :nsl]

                def alloc(name, l=l):
                    return coefp.tile([128, maxsl], BF16, tag=f"ct_{name}",
                                      name=f"ct_{l}_{name}")
                _coef_tree(nc, alloc, e_sb, cj, rinv, nsl)

                if l == 0:
                    for s0, ns in _chunks(NSLOT):
                        a_t = gath.tile([128, CHUNK_SLOTS, 32], BF16, tag="a1c")
                        b_t = gath.tile([128, CHUNK_SLOTS, 32], BF16, tag="b1c")
                        nc.sync.dma_start(a_t[:, :ns, :], a1_d[:, s0:s0 + ns, :])
                        nc.sync.dma_start(b_t[:, :ns, :], b1_d[:, s0:s0 + ns, :])
                        u_t = temps.tile([128, CHUNK_SLOTS, 16, 2], BF16, tag="u")
                        v_t = temps.tile([128, CHUNK_SLOTS, 16, 2], BF16, tag="v")

                        def cbc(j, s0=s0, ns=ns, cj=cj):
                            return (cj[j][:, s0:s0 + ns, :].unsqueeze(2)
                                    .to_broadcast([128, ns, 16, 2]))
                        uv = _gate(nc, a_t[:, :ns, :], b_t[:, :ns, :], cbc,
                                   u_t, v_t, ns)
                        hap = h1_dram.ap()
                        dst = hap[s0 * 128: s0 * 128 + ns * 128, :32]
                        dst = dst.rearrange("(c p) e -> p c e", p=128)
                        nc.sync.dma_start(dst, uv)

                elif l == 1:
                    ia_sb = persist.tile([128, n2rows // 16], I16, tag="ia")
                    ib_sb = persist.tile([128, n2rows // 16], I16, tag="ib")
                    nc.sync.dma_start(ia_sb[:], ia2[:])
                    nc.sync.dma_start(ib_sb[:], ib2[:])
                    src_ap = h1_dram[H_BASE:W]
                    for s0, ns in _chunks(nslot2):
                        a_t = gath.tile([128, CHUNK_SLOTS, E], BF16, tag="a")
                        b_t = gath.tile([128, CHUNK_SLOTS, E], BF16, tag="b")
                        col = s0 * 8
                        slot = 0
                        for n in _gathers(ns):
                            ncols = n // 16
                            nsg = n // 128
                            nc.gpsimd.dma_gather(
                                a_t[:, slot:slot + nsg, :], src_ap,
                                ia_sb[:, col:col + ncols], n, n, E)
                            nc.gpsimd.dma_gather(
                                b_t[:, slot:slot + nsg, :], src_ap,
                                ib_sb[:, col:col + ncols], n, n, E)
                            col += ncols
                            slot += nsg
                        u_t = temps.tile([128, CHUNK_SLOTS, 16, 2], BF16, tag="u")
                        v_t = temps.tile([128, CHUNK_SLOTS, 16, 2], BF16, tag="v")

                        def cbc(j, s0=s0, ns=ns, cj=cj):
                            return (cj[j][:, s0:s0 + ns, :].unsqueeze(2)
                                    .to_broadcast([128, ns, 16, 2]))
                        uv = _gate(nc, a_t[:, :ns, :32], b_t[:, :ns, :32], cbc,
                                   u_t, v_t, ns)
                        hap = h2_dram.ap()
                        dst = hap[s0 * 128: s0 * 128 + ns * 128, :]
                        dst = dst.rearrange("(c p) e -> p c e", p=128)
                        nc.sync.dma_start(dst, uv)

                else:
                    cjv = [c.rearrange("p (c j) d -> p c j d", j=7)
                           for c in cj]
                    hap = h2_dram.ap()
                    for c0, ncb in _chunks(nc3, CB3):
                        t3 = gath.tile([128, CB3, 256], BF16, tag="l3")
                        src = hap[c0 * 1024: (c0 + ncb) * 1024, :]
                        src = src.rearrange("(c p r) e -> p c (r e)",
                                            p=128, r=8)
                        nc.sync.dma_start(t3[:, :ncb, :], src)
                        for j in range(7):
                            u_t = temps.tile([128, CB3, 16, 2], BF16, tag="u3")
                            v_t = temps.tile([128, CB3, 16, 2], BF16, tag="v3")

                            def cbc(q, c0=c0, ncb=ncb, j=j, cjv=cjv):
                                return (cjv[q][:, c0:c0 + ncb, j, :]
                                        .unsqueeze(2)
                                        .to_broadcast([128, ncb, 16, 2]))
                            uv = _gate(nc, t3[:, :ncb, 32 * j:32 * j + 32],
                                       t3[:, :ncb, 32 * j + 32:32 * j + 64],
                                       cbc, u_t, v_t, ncb)
                            for c in range(ncb):
                                nc.tensor.matmul(
                                    psum_out[:],
                                    lhsT=g10_sb[:, c0 + c, j, :],
                                    rhs=uv[:, c, :],
                                    start=(mm_i == 0),
                                    stop=(mm_i == n_mm - 1),
                                )
                                mm_i += 1
            assert mm_i == n_mm

            out_sb = persist.tile([K, BC], F32, tag="outsb")
            nc.scalar.mul(out_sb[:], psum_out[:], 1.0 / TAU)
            nc.sync.dma_start(out_dram[:], out_sb[:])

    nc.compile()
    _NC_CACHE[key] = nc
    return nc


def _wrap(idx):
    n = idx.shape[0]
    arr = np.empty((128, n // 16), dtype=np.int16)
    blk = idx.reshape(n // 16, 16).T.astype(np.int16)
    for g in range(8):
        arr[g * 16:(g + 1) * 16, :] = blk
    return arr


def _build_cover(a3, b3):
    """Path cover of the layer-3 access multigraph (vertices = layer-2
    logical neurons, edge k = (a3[k], b3[k])), chopped into <=8-row
    segments and bin-packed into 8-row bins.

    Returns (rows2, slack, slot_neuron, slot_flip, nbins): rows2[r] =
    layer-2 logical neuron at h2 row r (0 for slack rows), slot_neuron
    [bin, j] = layer-3 neuron whose inputs are rows (8*bin+j, 8*bin+j+1)
    (-1 = garbage slot), slot_flip = a/b orientation flip."""
    Edg = len(a3)
    head = np.full(W, -1, dtype=np.int64)
    nxt = np.empty(2 * Edg, dtype=np.int64)
    for k in range(Edg):
        u, v = a3[k], b3[k]
        nxt[2 * k] = head[u]
        head[u] = 2 * k          # half 2k stored at a-side (u -> v)
        nxt[2 * k + 1] = head[v]
        head[v] = 2 * k + 1      # half 2k+1 stored at b-side (v -> u)
    used = np.zeros(Edg, dtype=bool)
    deg = np.bincount(np.concatenate([a3, b3]), minlength=W)

    def walk(start):
        verts = [start]
        edges = []
        v = start
        while True:
            h = head[v]
            while h != -1 and used[h >> 1]:
                h = nxt[h]
            head[v] = h
            if h == -1:
                break
            k = int(h) >> 1
            used[k] = True
            if h & 1:            # stored at b-side: traversal b -> a
                w = int(a3[k])
                edges.append((k, True))
            else:
                w = int(b3[k])
                edges.append((k, False))
            verts.append(w)
            v = w
        return verts, edges

    paths = []
    for s in np.nonzero(deg % 2 == 1)[0]:
        verts, edges = walk(int(s))
        if edges:
            paths.append((verts, edges))
    for s in range(W):
        while True:
            verts, edges = walk(s)
            if not edges:
                break
            paths.append((verts, edges))
    assert used.all()

    # chop into segments of <= 8 rows (7 edges); cut vertex shared
    segs = []
    for verts, edges in paths:
        i = 0
        while i < len(edges):
            j = min(i + 7, len(edges))
            segs.append((verts[i:j + 1], edges[i:j]))
            i = j

    # bin-pack segments (2..8 rows) into 8-row bins
    from collections import defaultdict
    by_size = defaultdict(list)
    for sg in segs:
        by_size[len(sg[0])].append(sg)

    def pop_fit(cap):
        for s in range(cap, 1, -1):
            if by_size[s]:
                return by_size[s].pop()
        return None

    bins = []
    while True:
        sg = pop_fit(8)
        if sg is None:
            break
        cur = [sg]
        cap = 8 - len(sg[0])
        while cap >= 2:
            sg2 = pop_fit(cap)
            if sg2 is None:
                break
            cur.append(sg2)
            cap -= len(sg2[0])
        bins.append(cur)

    nbins = (len(bins) + 127) // 128 * 128
    while len(bins) < nbins:
        bins.append([])

    rows2 = np.zeros(nbins * 8, dtype=np.int64)
    slack = np.ones(nbins * 8, dtype=bool)
    slot_neuron = np.full((nbins, 7), -1, dtype=np.int64)
    slot_flip = np.zeros((nbins, 7), dtype=bool)
    for bi, bin_segs in enumerate(bins):
        r = 0
        for verts, edges in bin_segs:
            for q, v in enumerate(verts):
                rows2[bi * 8 + r + q] = v
                slack[bi * 8 + r + q] = False
            for q, (k, flip) in enumerate(edges):
                slot_neuron[bi, r + q] = k
                slot_flip[bi, r + q] = flip
            r += len(verts)
    return rows2, slack, slot_neuron, slot_flip, nbins


def kernel(x, w1, w2, w3, idx_a1, idx_b1, idx_a2, idx_b2, idx_a3, idx_b3):
    x = np.asarray(x, dtype=np.float32)
    ws = [np.asarray(w, dtype=np.float32) for w in (w1, w2, w3)]
    ias = [np.asarray(i).astype(np.int64) for i in (idx_a1, idx_a2, idx_a3)]
    ibs = [np.asarray(i).astype(np.int64) for i in (idx_b1, idx_b2, idx_b3)]

    rows2, slack, slot_neuron, slot_flip, nbins = _build_cover(ias[2], ibs[2])
    n2rows = nbins * 8
    nc3 = nbins // 128

    # layer-2 instance gather lists (h1 rows in logical order, rebased)
    ia2i = ias[1][rows2] - H_BASE
    ib2i = ibs[1][rows2] - H_BASE
    ia2i[slack] = 0
    ib2i[slack] = 0

    # trailing-idx fix: every 1024-idx sublist must end with idxs >= 0 in
    # both lists (SWDGE trims trailing negatives). The sublist-final row is
    # row 7 of bins 128k+127; reorder bins to place "good" bins there.
    good = ((ia2i.reshape(-1, 8)[:, 7] >= 0)
            & (ib2i.reshape(-1, 8)[:, 7] >= 0))
    order = np.arange(nbins)
    bad_bound = [kk for kk in range(127, nbins, 128) if not good[kk]]
    spare = [int(g) for g in np.nonzero(good)[0] if (g % 128) != 127]
    assert len(spare) >= len(bad_bound), "not enough good bins"
    for bb, sp in zip(bad_bound, spare):
        order[bb], order[sp] = order[sp], order[bb]

    rows2 = rows2.reshape(nbins, 8)[order].reshape(-1)
    slack = slack.reshape(nbins, 8)[order].reshape(-1)
    ia2i = ia2i.reshape(nbins, 8)[order].reshape(-1)
    ib2i = ib2i.reshape(nbins, 8)[order].reshape(-1)
    slot_neuron = slot_neuron[order]
    slot_flip = slot_flip[order]
    assert ((ia2i.reshape(-1, 8)[127::128, 7] >= 0).all()
            and (ib2i.reshape(-1, 8)[127::128, 7] >= 0).all())

    nc = build_nc(nbins)

    shared = {"ia2": _wrap(ia2i), "ib2": _wrap(ib2i)}
    # weights: layer 1 logical, layer 2 per-instance, layer 3 per-slot
    wf2 = ws[1][rows2]
    sn = slot_neuron.reshape(-1)
    sf = slot_flip.reshape(-1)
    valid = sn >= 0
    wf3 = np.zeros((nbins * 7, 16), dtype=np.float32)   # bin-major (bin*7+j)
    wf3[valid] = ws[2][sn[valid]]
    fl = valid & sf
    wf3[fl] = wf3[fl][:, SWP]
    for l, (wfl, nsl) in enumerate(
            [(ws[0], NSLOT), (wf2, n2rows // 128)]):
        shared[f"wf{l}"] = np.ascontiguousarray(
            wfl.reshape(nsl, 128, 16).transpose(1, 2, 0)
        ).astype(ml_dtypes.bfloat16)
    # layer-3 coef slots are (c, j)-flattened per partition: [p, g, c, j]
    shared["wf2"] = np.ascontiguousarray(
        wf3.reshape(nc3, 128, 7, 16).transpose(1, 3, 0, 2)
        .reshape(128, 16, nc3 * 7)
    ).astype(ml_dtypes.bfloat16)

    g10 = np.zeros((nbins * 7, K), dtype=np.float32)
    g10[valid, sn[valid] // (W // K)] = 1.0
    shared["g10"] = np.ascontiguousarray(
        g10.reshape(nc3, 128, 7, K).transpose(1, 0, 2, 3)
    ).astype(ml_dtypes.bfloat16)

    # host-gathered layer-1 inputs
    ga = x[:, ias[0]]
    gb = x[:, ibs[0]]
    in_maps = []
    for c in range(NCORES):
        sl = slice(c * BC, (c + 1) * BC)
        m = dict(shared)
        for nm, g in (("a1", ga), ("b1", gb)):
            arr = g[sl].T.reshape(NSLOT, 128, BC).transpose(1, 0, 2)
            m[nm] = np.ascontiguousarray(arr).astype(ml_dtypes.bfloat16)
        in_maps.append(m)

    res = run_bass_kernel_spmd(nc, in_maps, core_ids=list(range(NCORES)))

    out = np.empty((BATCH, K), dtype=np.float32)
    for c in range(NCORES):
        out[c * BC:(c + 1) * BC] = res.results[c]["out"].T
    return out


# revision 17
# speedup vs baseline: 1.5381x; 1.0410x over previous
"""DiffLogic network TRN2 kernel: 3 logic layers [B=256, W=64000] + GroupSum.

Sharding: pure data-parallel over batch across 8 cores (B=32/core), no
inter-core communication.

Layer 1: inputs host-gathered (x[:, idx] is pure indexing), no device
gather. Layer 2: SWDGE dma_gather of 256B rows from h1. Layer 3: NO
gather at all — layer 2's consumer stream is ordered as a path cover of
layer 3's access graph, so h2 (64B rows) already contains every layer-3
input pair at adjacent rows; layer 3 just bulk-loads h2 sequentially and
evaluates gates on the 7 adjacent-row offsets of each 8-row block.

Softmax coefs: exp (ACT) + shared-subexpression add/sub tree over the 16
gate planes (all bf16 TensorTensor in 2x mode) instead of TensorReduce.
Gate coefs stored duplicated-in-pairs ([.., 2]) so the batch-broadcast
AP keeps an innermost stride-1 pair and every gate op runs in 2x mode.
GroupSum via PE one-hot matmul accumulation in PSUM.
"""
import numpy as np
import ml_dtypes

import concourse.bass as bass
import concourse.tile as tile
import concourse.bacc as bacc
import concourse.mybir as mybir
from concourse.bass_utils import run_bass_kernel_spmd
from concourse.library_config import mlp

W = 64000
BATCH = 256
NCORES = 8
BC = BATCH // NCORES        # 32 batch rows per core
IN_DIM = 1024
K = 10
TAU = 30.0
NSLOT = W // 128            # 500
E = 128                     # bf16 elements per h1 row (256B); [:32] real
CHUNK_SLOTS = 48
GPN = 1024                  # idxs per dma_gather instruction
H_BASE = 32000              # gather base row (signed int16 rebase)
CB3 = 16                    # layer-3 block-columns per chunk

# a<->b swap permutation of the 16 gates: gate g(b,a) == gate SWP[g](a,b)
SWP = np.array([0, 1, 4, 5, 2, 3, 6, 7, 8, 9, 12, 13, 10, 11, 14, 15])

BF16 = mybir.dt.bfloat16
F32 = mybir.dt.float32
I16 = mybir.dt.int16
ADD = mybir.AluOpType.add
SUB = mybir.AluOpType.subtract

_NC_CACHE = {}


def _chunks(nslot, step=CHUNK_SLOTS):
    s = 0
    while s < nslot:
        n = min(step, nslot - s)
        yield s, n
        s += n


def _gathers(nslots):
    n = nslots * 128
    out = []
    while n > 0:
        g = min(GPN, n)
        out.append(g)
        n -= g
    return out


def _coef_tree(nc, alloc, e_sb, cj, rinv_dst, nsl):
    """craw0..3 + ssum from the 16 exp gate planes via shared add/sub
    subexpressions; normalized into duplicated-pair coef tiles."""

    def pl(g):
        return e_sb[:, g, :]

    tmp = {}

    def mk(name, eng, op, x, y):
        slot = SLOT[name]
        dst = alloc(ALIAS.get(slot, slot))
        eng.tensor_tensor(out=dst[:, :nsl], in0=x, in1=y, op=op)
        tmp[name] = dst
        return dst[:, :nsl]

    def t(name):
        return tmp[name][:, :nsl]

    # names mapped to shared buffer slots (liveness-based reuse)
    # an op's out slot must never alias one of its own input slots
    SLOT = {"s01": "A", "s23": "B", "s45": "C", "s67": "D", "s89": "E",
            "sAB": "F", "sCD": "G", "sEF": "H",
            "q0": "L", "q1": "M", "q2": "N", "q3": "O",
            "c0": "I", "h01": "J", "ssum": "K",
            "c1a": "A2", "c1b": "C2", "c1": "F2", "c2": "D2",
            "t1": "P1", "t2": "P2", "t3": "P3", "t4": "P4", "t5": "P5",
            "t5d": "P6", "u1": "P7", "u2": "P1b", "u3": "P2b", "c3": "P3b"}
    ALIAS = {"A2": "A", "C2": "C", "F2": "F", "D2": "D",
             "P1b": "P1", "P2b": "P2", "P3b": "P3"}

    dve, po = nc.vector, nc.gpsimd
    for i, nm in enumerate(["s01", "s23", "s45", "s67", "s89", "sAB", "sCD", "sEF"]):
        mk(nm, dve, ADD, pl(2 * i), pl(2 * i + 1))
    mk("q0", dve, ADD, t("s01"), t("s23"))
    mk("q1", dve, ADD, t("s45"), t("s67"))
    mk("q2", dve, ADD, t("s89"), t("sAB"))
    mk("q3", dve, ADD, t("sCD"), t("sEF"))
    mk("c0", dve, ADD, t("q2"), t("q3"))
    mk("h01", dve, ADD, t("q0"), t("q1"))
    mk("ssum", dve, ADD, t("h01"), t("c0"))
    mk("c1a", dve, ADD, t("s23"), t("s67"))
    mk("c1b", dve, ADD, t("s89"), t("sCD"))
    mk("c1", dve, SUB, t("c1a"), t("c1b"))
    mk("c2", dve, SUB, t("q1"), t("q2"))
    # c3 = e1-e2-e4-2e6-e7+e8+2e9+e11+e13-e14 (Pool, offloads DVE)
    mk("t1", po, SUB, pl(1), pl(2))
    mk("t2", po, SUB, pl(8), pl(4))
    mk("t3", po, SUB, pl(11), pl(7))
    mk("t4", po, SUB, pl(13), pl(14))
    mk("t5", po, SUB, pl(9), pl(6))
    mk("t5d", po, ADD, t("t5"), t("t5"))
    mk("u1", po, ADD, t("t1"), t("t2"))
    mk("u2", po, ADD, t("t3"), t("t4"))
    mk("u3", po, ADD, t("u1"), t("u2"))
    mk("c3", po, ADD, t("u3"), t("t5d"))

    nc.vector.reciprocal(out=rinv_dst, in_=t("ssum"))
    for j, nm in enumerate(["c0", "c1", "c2", "c3"]):
        eng = dve if j < 2 else po
        for kdup in range(2):
            eng.tensor_tensor(out=cj[j][:, :, kdup], in0=t(nm),
                              in1=rinv_dst, op=mybir.AluOpType.mult)


def _gate(nc, av, bv, cbc, u_t, v_t, ns):
    """h = (c3*b + c1)*a + (c2*b + c0), written into u_t[:, :ns].
    av/bv: [128, ns, 32] APs; cbc(j): coef broadcast [128, ns, 16, 2]."""
    av4 = av.rearrange("p n (a b) -> p n a b", b=2)
    bv4 = bv.rearrange("p n (a b) -> p n a b", b=2)
    u4 = u_t[:, :ns, :, :]
    v4 = v_t[:, :ns, :, :]
    nc.vector.tensor_mul(u4, bv4, cbc(3))
    nc.vector.tensor_add(u4, u4, cbc(1))
    nc.vector.tensor_mul(u4, u4, av4)
    nc.vector.tensor_mul(v4, bv4, cbc(2))
    nc.vector.tensor_add(v4, v4, cbc(0))
    nc.vector.tensor_add(u4, u4, v4)
    return u_t[:, :ns, :, :].rearrange("p n a b -> p n (a b)")


def build_nc(nbins=None):
    if nbins is None:
        nbins = _NC_CACHE["last_nbins"]
    _NC_CACHE["last_nbins"] = nbins
    key = ("nc", nbins)
    if key in _NC_CACHE:
        return _NC_CACHE[key]
    n2rows = nbins * 8
    nslot2 = n2rows // 128
    nc3 = nbins // 128          # block-columns for layer 3
    nslot3 = nc3 * 7            # coef slots for layer 3 (c, j)-flattened

    nc = bacc.Bacc("TRN2", target_bir_lowering=False, debug=False,
                   enable_asserts=False, num_devices=NCORES)

    a1_d = nc.dram_tensor("a1", [128, NSLOT, 32], BF16, kind="ExternalInput")
    b1_d = nc.dram_tensor("b1", [128, NSLOT, 32], BF16, kind="ExternalInput")
    nslots = [NSLOT, nslot2, nslot3]
    wf = [nc.dram_tensor(f"wf{l}", [128, 16, nslots[l]], BF16,
                         kind="ExternalInput") for l in range(3)]
    ia2 = nc.dram_tensor("ia2", [128, n2rows // 16], I16, kind="ExternalInput")
    ib2 = nc.dram_tensor("ib2", [128, n2rows // 16], I16, kind="ExternalInput")
    g10 = nc.dram_tensor("g10", [128, nc3, 7, K], BF16, kind="ExternalInput")
    h1_dram = nc.dram_tensor("h1", [W, E], BF16, kind="Internal")
    h2_dram = nc.dram_tensor("h2", [n2rows, 32], BF16, kind="Internal")
    out_dram = nc.dram_tensor("out", [K, BC], F32, kind="ExternalOutput")

    with tile.TileContext(nc) as tc:
        with (
            tc.tile_pool(name="persist", bufs=1) as persist,
            tc.tile_pool(name="coef", bufs=1) as coefp,
            tc.tile_pool(name="gath", bufs=2) as gath,
            tc.tile_pool(name="temps", bufs=2) as temps,
            tc.tile_pool(name="psum", bufs=1, space="PSUM") as psump,
        ):
            nc.gpsimd.load_library(mlp)

            g10_sb = persist.tile([128, nc3, 7, K], BF16, tag="g10")
            nc.sync.dma_start(g10_sb[:], g10[:])

            psum_out = psump.tile([K, BC], F32, tag="acc")
            n_mm = nc3 * 7
            mm_i = 0

            maxsl = max(nslots)
            for l in range(3):
                nsl = nslots[l]
                wf_t = coefp.tile([128, 16, maxsl], BF16, tag="wf",
                                  name=f"wf_sb{l}")
                wf_sb = wf_t[:, :, :nsl]
                nc.sync.dma_start(wf_sb, wf[l][:])
                e_t = coefp.tile([128, 16, maxsl], BF16, tag="e",
                                 name=f"e_sb{l}")
                e_sb = e_t[:, :, :nsl]
                nc.scalar.activation(e_sb, wf_sb,
                                     mybir.ActivationFunctionType.Exp)
                cj_t = [coefp.tile([128, maxsl, 2], BF16, tag=f"c{j}",
                                   name=f"cj{l}{j}") for j in range(4)]
                cj = [t[:, :nsl, :] for t in cj_t]
                rinv_t = coefp.tile([128, maxsl], F32, tag="rinv",
                                    name=f"rinv{l}")
                rinv = rinv_t[:, :nsl]

                def alloc(name, l=l):
                    return coefp.tile([128, maxsl], BF16, tag=f"ct_{name}",
                                      name=f"ct_{l}_{name}")
                _coef_tree(nc, alloc, e_sb, cj, rinv, nsl)

                if l == 0:
                    for s0, ns in _chunks(NSLOT):
                        a_t = gath.tile([128, CHUNK_SLOTS, 32], BF16, tag="a1c")
                        b_t = gath.tile([128, CHUNK_SLOTS, 32], BF16, tag="b1c")
                        nc.sync.dma_start(a_t[:, :ns, :], a1_d[:, s0:s0 + ns, :])
                        nc.sync.dma_start(b_t[:, :ns, :], b1_d[:, s0:s0 + ns, :])
                        u_t = temps.tile([128, CHUNK_SLOTS, 16, 2], BF16, tag="u")
                        v_t = temps.tile([128, CHUNK_SLOTS, 16, 2], BF16, tag="v")

                        def cbc(j, s0=s0, ns=ns, cj=cj):
                            return (cj[j][:, s0:s0 + ns, :].unsqueeze(2)
                                    .to_broadcast([128, ns, 16, 2]))
                        uv = _gate(nc, a_t[:, :ns, :], b_t[:, :ns, :], cbc,
                                   u_t, v_t, ns)
                        hap = h1_dram.ap()
                        dst = hap[s0 * 128: s0 * 128 + ns * 128, :32]
                        dst = dst.rearrange("(c p) e -> p c e", p=128)
                        nc.sync.dma_start(dst, uv)

                elif l == 1:
                    ia_sb = persist.tile([128, n2rows // 16], I16, tag="ia")
                    ib_sb = persist.tile([128, n2rows // 16], I16, tag="ib")
                    nc.sync.dma_start(ia_sb[:], ia2[:])
                    nc.sync.dma_start(ib_sb[:], ib2[:])
                    src_ap = h1_dram[H_BASE:W]
                    for s0, ns in _chunks(nslot2):
                        a_t = gath.tile([128, CHUNK_SLOTS, E], BF16, tag="a")
                        b_t = gath.tile([128, CHUNK_SLOTS, E], BF16, tag="b")
                        col = s0 * 8
                        slot = 0
                        for n in _gathers(ns):
                            ncols = n // 16
                            nsg = n // 128
                            nc.gpsimd.dma_gather(
                                a_t[:, slot:slot + nsg, :], src_ap,
                                ia_sb[:, col:col + ncols], n, n, E)
                            nc.gpsimd.dma_gather(
                                b_t[:, slot:slot + nsg, :], src_ap,
                                ib_sb[:, col:col + ncols], n, n, E)
                            col += ncols
                            slot += nsg
                        u_t = temps.tile([128, CHUNK_SLOTS, 16, 2], BF16, tag="u")
                        v_t = temps.tile([128, CHUNK_SLOTS, 16, 2], BF16, tag="v")

                        def cbc(j, s0=s0, ns=ns, cj=cj):
                            return (cj[j][:, s0:s0 + ns, :].unsqueeze(2)
                                    .to_broadcast([128, ns, 16, 2]))
                        uv = _gate(nc, a_t[:, :ns, :32], b_t[:, :ns, :32], cbc,
                                   u_t, v_t, ns)
                        # p-major row map: stream (p, c) -> row p*nslot2+c,
                        # so the write is per-partition contiguous
                        hap = h2_dram.ap()
                        dst = hap.rearrange("(p c) e -> p c e", p=128)
                        nc.sync.dma_start(dst[:, s0:s0 + ns, :], uv)

                else:
                    cjv = [c.rearrange("p (c j) d -> p c j d", j=7)
                           for c in cj]
                    hap = h2_dram.ap()
                    for c0, ncb in _chunks(nc3, CB3):
                        t3 = gath.tile([128, CB3, 256], BF16, tag="l3")
                        src = hap[c0 * 1024: (c0 + ncb) * 1024, :]
                        src = src.rearrange("(c p r) e -> p c (r e)",
                                            p=128, r=8)
                        nc.sync.dma_start(t3[:, :ncb, :], src)
                        for j in range(7):
                            u_t = temps.tile([128, CB3, 16, 2], BF16, tag="u3")
                            v_t = temps.tile([128, CB3, 16, 2], BF16, tag="v3")

                            def cbc(q, c0=c0, ncb=ncb, j=j, cjv=cjv):
                                return (cjv[q][:, c0:c0 + ncb, j, :]
                                        .unsqueeze(2)
                                        .to_broadcast([128, ncb, 16, 2]))
                            uv = _gate(nc, t3[:, :ncb, 32 * j:32 * j + 32],
                                       t3[:, :ncb, 32 * j + 32:32 * j + 64],
                                       cbc, u_t, v_t, ncb)
                            for c in range(ncb):
                                nc.tensor.matmul(
                                    psum_out[:],
                                    lhsT=g10_sb[:, c0 + c, j, :],
                                    rhs=uv[:, c, :],
                                    start=(mm_i == 0),
                                    stop=(mm_i == n_mm - 1),
                                )
                                mm_i += 1
            assert mm_i == n_mm

            out_sb = persist.tile([K, BC], F32, tag="outsb")
            nc.scalar.mul(out_sb[:], psum_out[:], 1.0 / TAU)
            nc.sync.dma_start(out_dram[:], out_sb[:])

    nc.compile()
    _NC_CACHE[key] = nc
    return nc


def _wrap(idx):
    n = idx.shape[0]
    arr = np.empty((128, n // 16), dtype=np.int16)
    blk = idx.reshape(n // 16, 16).T.astype(np.int16)
    for g in range(8):
        arr[g * 16:(g + 1) * 16, :] = blk
    return arr


def _build_cover(a3, b3):
    """Path cover of the layer-3 access multigraph (vertices = layer-2
    logical neurons, edge k = (a3[k], b3[k])), chopped into <=8-row
    segments and bin-packed into 8-row bins.

    Returns (rows2, slack, slot_neuron, slot_flip, nbins): rows2[r] =
    layer-2 logical neuron at h2 row r (0 for slack rows), slot_neuron
    [bin, j] = layer-3 neuron whose inputs are rows (8*bin+j, 8*bin+j+1)
    (-1 = garbage slot), slot_flip = a/b orientation flip."""
    Edg = len(a3)
    head = np.full(W, -1, dtype=np.int64)
    nxt = np.empty(2 * Edg, dtype=np.int64)
    for k in range(Edg):
        u, v = a3[k], b3[k]
        nxt[2 * k] = head[u]
        head[u] = 2 * k          # half 2k stored at a-side (u -> v)
        nxt[2 * k + 1] = head[v]
        head[v] = 2 * k + 1      # half 2k+1 stored at b-side (v -> u)
    used = np.zeros(Edg, dtype=bool)
    deg = np.bincount(np.concatenate([a3, b3]), minlength=W)

    def walk(start):
        verts = [start]
        edges = []
        v = start
        while True:
            h = head[v]
            while h != -1 and used[h >> 1]:
                h = nxt[h]
            head[v] = h
            if h == -1:
                break
            k = int(h) >> 1
            used[k] = True
            if h & 1:            # stored at b-side: traversal b -> a
                w = int(a3[k])
                edges.append((k, True))
            else:
                w = int(b3[k])
                edges.append((k, False))
            verts.append(w)
            v = w
        return verts, edges

    paths = []
    for s in np.nonzero(deg % 2 == 1)[0]:
        verts, edges = walk(int(s))
        if edges:
            paths.append((verts, edges))
    for s in range(W):
        while True:
            verts, edges = walk(s)
            if not edges:
                break
            paths.append((verts, edges))
    assert used.all()

    # chop into segments of <= 8 rows (7 edges); cut vertex shared
    segs = []
    for verts, edges in paths:
        i = 0
        while i < len(edges):
            j = min(i + 7, len(edges))
            segs.append((verts[i:j + 1], edges[i:j]))
            i = j

    # bin-pack segments (2..8 rows) into 8-row bins
    from collections import defaultdict
    by_size = defaultdict(list)
    for sg in segs:
        by_size[len(sg[0])].append(sg)

    def pop_fit(cap):
        for s in range(cap, 1, -1):
            if by_size[s]:
                return by_size[s].pop()
        return None

    bins = []
    while True:
        sg = pop_fit(8)
        if sg is None:
            break
        cur = [sg]
        cap = 8 - len(sg[0])
        while cap >= 2:
            sg2 = pop_fit(cap)
            if sg2 is None:
                break
            cur.append(sg2)
            cap -= len(sg2[0])
        bins.append(cur)

    nbins = (len(bins) + 127) // 128 * 128
    while len(bins) < nbins:
        bins.append([])

    rows2 = np.zeros(nbins * 8, dtype=np.int64)
    slack = np.ones(nbins * 8, dtype=bool)
    slot_neuron = np.full((nbins, 7), -1, dtype=np.int64)
    slot_flip = np.zeros((nbins, 7), dtype=bool)
    for bi, bin_segs in enumerate(bins):
        r = 0
        for verts, edges in bin_segs:
            for q, v in enumerate(verts):
                rows2[bi * 8 + r + q] = v
                slack[bi * 8 + r + q] = False
            for q, (k, flip) in enumerate(edges):
                slot_neuron[bi, r + q] = k
                slot_flip[bi, r + q] = flip
            r += len(verts)
    return rows2, slack, slot_neuron, slot_flip, nbins


def kernel(x, w1, w2, w3, idx_a1, idx_b1, idx_a2, idx_b2, idx_a3, idx_b3):
    x = np.asarray(x, dtype=np.float32)
    ws = [np.asarray(w, dtype=np.float32) for w in (w1, w2, w3)]
    ias = [np.asarray(i).astype(np.int64) for i in (idx_a1, idx_a2, idx_a3)]
    ibs = [np.asarray(i).astype(np.int64) for i in (idx_b1, idx_b2, idx_b3)]

    rows2, slack, slot_neuron, slot_flip, nbins = _build_cover(ias[2], ibs[2])
    n2rows = nbins * 8
    nc3 = nbins // 128

    # layer-2 instance gather lists (h1 rows in logical order, rebased)
    ia2i = ias[1][rows2] - H_BASE
    ib2i = ibs[1][rows2] - H_BASE
    ia2i[slack] = 0
    ib2i[slack] = 0

    # trailing-idx fix: every 1024-idx gather sublist must end with idxs
    # >= 0 in both lists (SWDGE trims trailing negatives). With the
    # p-major h2 write (stream (p,c) -> row p*nslot2+c), stream position
    # 1024k+1023 is row 7 of bin B0+k where B0 = nbins - nbins//128.
    # Reorder bins so the tail block holds only "good" bins.
    nslot2 = n2rows // 128
    good = ((ia2i.reshape(-1, 8)[:, 7] >= 0)
            & (ib2i.reshape(-1, 8)[:, 7] >= 0))
    B0 = nbins - nbins // 128
    order = np.arange(nbins)
    bad_bound = [kk for kk in range(B0, nbins) if not good[kk]]
    spare = [int(g) for g in np.nonzero(good)[0] if g < B0]
    assert len(spare) >= len(bad_bound), "not enough good bins"
    for bb, sp in zip(bad_bound, spare):
        order[bb], order[sp] = order[sp], order[bb]

    rows2 = rows2.reshape(nbins, 8)[order].reshape(-1)
    slack = slack.reshape(nbins, 8)[order].reshape(-1)
    ia2i = ia2i.reshape(nbins, 8)[order].reshape(-1)
    ib2i = ib2i.reshape(nbins, 8)[order].reshape(-1)
    slot_neuron = slot_neuron[order]
    slot_flip = slot_flip[order]

    # stream relabel for the p-major write
    n_arr = np.arange(n2rows)
    r_of_n = (n_arr % 128) * nslot2 + n_arr // 128
    ia2i = ia2i[r_of_n]
    ib2i = ib2i[r_of_n]
    assert (ia2i[1023::1024] >= 0).all() and (ib2i[1023::1024] >= 0).all()

    nc = build_nc(nbins)

    shared = {"ia2": _wrap(ia2i), "ib2": _wrap(ib2i)}
    # weights: layer 1 logical, layer 2 per-instance, layer 3 per-slot
    wf2 = ws[1][rows2[r_of_n]]
    sn = slot_neuron.reshape(-1)
    sf = slot_flip.reshape(-1)
    valid = sn >= 0
    wf3 = np.zeros((nbins * 7, 16), dtype=np.float32)   # bin-major (bin*7+j)
    wf3[valid] = ws[2][sn[valid]]
    fl = valid & sf
    wf3[fl] = wf3[fl][:, SWP]
    for l, (wfl, nsl) in enumerate(
            [(ws[0], NSLOT), (wf2, n2rows // 128)]):
        shared[f"wf{l}"] = np.ascontiguousarray(
            wfl.reshape(nsl, 128, 16).transpose(1, 2, 0)
        ).astype(ml_dtypes.bfloat16)
    # layer-3 coef slots are (c, j)-flattened per partition: [p, g, c, j]
    shared["wf2"] = np.ascontiguousarray(
        wf3.reshape(nc3, 128, 7, 16).transpose(1, 3, 0, 2)
        .reshape(128, 16, nc3 * 7)
    ).astype(ml_dtypes.bfloat16)

    g10 = np.zeros((nbins * 7, K), dtype=np.float32)
    g10[valid, sn[valid] // (W // K)] = 1.0
    shared["g10"] = np.ascontiguousarray(
        g10.reshape(nc3, 128, 7, K).transpose(1, 0, 2, 3)
    ).astype(ml_dtypes.bfloat16)

    # host-gathered layer-1 inputs
    ga = x[:, ias[0]]
    gb = x[:, ibs[0]]
    in_maps = []
    for c in range(NCORES):
        sl = slice(c * BC, (c + 1) * BC)
        m = dict(shared)
        for nm, g in (("a1", ga), ("b1", gb)):
            arr = g[sl].T.reshape(NSLOT, 128, BC).transpose(1, 0, 2)
            m[nm] = np.ascontiguousarray(arr).astype(ml_dtypes.bfloat16)
        in_maps.append(m)

    res = run_bass_kernel_spmd(nc, in_maps, core_ids=list(range(NCORES)))

    out = np.empty((BATCH, K), dtype=np.float32)
    for c in range(NCORES):
        out[c * BC:(c + 1) * BC] = res.results[c]["out"].T
    return out


# revision 22
# speedup vs baseline: 1.6279x; 1.0584x over previous
"""DiffLogic network TRN2 kernel: 3 logic layers [B=256, W=64000] + GroupSum.

Sharding: pure data-parallel over batch across 8 cores (B=32/core), no
inter-core communication.

Layer 1: inputs host-gathered (x[:, idx] is pure indexing), no device
gather. Layer 2: SWDGE dma_gather of 256B rows from h1. Layer 3: NO
gather at all — layer 2's consumer stream is ordered as a path cover of
layer 3's access graph, so h2 (64B rows) already contains every layer-3
input pair at adjacent rows; layer 3 just bulk-loads h2 sequentially and
evaluates gates on the 7 adjacent-row offsets of each 8-row block.

Softmax coefs: exp (ACT) + shared-subexpression add/sub tree over the 16
gate planes (all bf16 TensorTensor in 2x mode) instead of TensorReduce.
Gate coefs stored duplicated-in-pairs ([.., 2]) so the batch-broadcast
AP keeps an innermost stride-1 pair and every gate op runs in 2x mode.
GroupSum via PE one-hot matmul accumulation in PSUM.
"""
import numpy as np
import ml_dtypes

import concourse.bass as bass
import concourse.tile as tile
import concourse.bacc as bacc
import concourse.mybir as mybir
from concourse.bass_utils import run_bass_kernel_spmd
from concourse.library_config import mlp

W = 64000
BATCH = 256
NCORES = 8
BC = BATCH // NCORES        # 32 batch rows per core
IN_DIM = 1024
K = 10
TAU = 30.0
NSLOT = W // 128            # 500
E = 128                     # bf16 elements per h1 row (256B); [:32] real
CHUNK_SLOTS = 40
GPN = 1024                  # idxs per dma_gather instruction
H_BASE = 32000              # gather base row (signed int16 rebase)
CB3 = 16                    # layer-3 block-columns per chunk

# a<->b swap permutation of the 16 gates: gate g(b,a) == gate SWP[g](a,b)
SWP = np.array([0, 1, 4, 5, 2, 3, 6, 7, 8, 9, 12, 13, 10, 11, 14, 15])

BF16 = mybir.dt.bfloat16
F32 = mybir.dt.float32
I16 = mybir.dt.int16
ADD = mybir.AluOpType.add
SUB = mybir.AluOpType.subtract

_NC_CACHE = {}


def _chunks(nslot, step=CHUNK_SLOTS):
    s = 0
    while s < nslot:
        n = min(step, nslot - s)
        yield s, n
        s += n


def _gathers(nslots):
    n = nslots * 128
    out = []
    while n > 0:
        g = min(GPN, n)
        out.append(g)
        n -= g
    return out


def _coef_tree(nc, alloc, e_sb, cj, rinv_dst, nsl):
    """craw0..3 + ssum from the 16 exp gate planes via shared add/sub
    subexpressions; normalized into duplicated-pair coef tiles."""

    def pl(g):
        return e_sb[:, g, :]

    tmp = {}

    def mk(name, eng, op, x, y):
        slot = SLOT[name]
        dst = alloc(ALIAS.get(slot, slot))
        eng.tensor_tensor(out=dst[:, :nsl], in0=x, in1=y, op=op)
        tmp[name] = dst
        return dst[:, :nsl]

    def t(name):
        return tmp[name][:, :nsl]

    # names mapped to shared buffer slots (liveness-based reuse)
    # an op's out slot must never alias one of its own input slots
    SLOT = {"s01": "A", "s23": "B", "s45": "C", "s67": "D", "s89": "E",
            "sAB": "F", "sCD": "G", "sEF": "H",
            "q0": "L", "q1": "M", "q2": "N", "q3": "O",
            "c0": "I", "h01": "J", "ssum": "K",
            "c1a": "A2", "c1b": "C2", "c1": "F2", "c2": "D2",
            "t1": "P1", "t2": "P2", "t3": "P3", "t4": "P4", "t5": "P5",
            "t5d": "P6", "u1": "P7", "u2": "P1b", "u3": "P2b", "c3": "P3b"}
    ALIAS = {"A2": "A", "C2": "C", "F2": "F", "D2": "D",
             "P1b": "P1", "P2b": "P2", "P3b": "P3"}

    dve, po = nc.vector, nc.gpsimd
    for i, nm in enumerate(["s01", "s23", "s45", "s67", "s89", "sAB", "sCD", "sEF"]):
        mk(nm, dve, ADD, pl(2 * i), pl(2 * i + 1))
    mk("q0", dve, ADD, t("s01"), t("s23"))
    mk("q1", dve, ADD, t("s45"), t("s67"))
    mk("q2", dve, ADD, t("s89"), t("sAB"))
    mk("q3", dve, ADD, t("sCD"), t("sEF"))
    mk("c0", dve, ADD, t("q2"), t("q3"))
    mk("h01", dve, ADD, t("q0"), t("q1"))
    mk("ssum", dve, ADD, t("h01"), t("c0"))
    mk("c1a", dve, ADD, t("s23"), t("s67"))
    mk("c1b", dve, ADD, t("s89"), t("sCD"))
    mk("c1", dve, SUB, t("c1a"), t("c1b"))
    mk("c2", dve, SUB, t("q1"), t("q2"))
    # c3 = e1-e2-e4-2e6-e7+e8+2e9+e11+e13-e14 (Pool, offloads DVE)
    mk("t1", po, SUB, pl(1), pl(2))
    mk("t2", po, SUB, pl(8), pl(4))
    mk("t3", po, SUB, pl(11), pl(7))
    mk("t4", po, SUB, pl(13), pl(14))
    mk("t5", po, SUB, pl(9), pl(6))
    mk("t5d", po, ADD, t("t5"), t("t5"))
    mk("u1", po, ADD, t("t1"), t("t2"))
    mk("u2", po, ADD, t("t3"), t("t4"))
    mk("u3", po, ADD, t("u1"), t("u2"))
    mk("c3", po, ADD, t("u3"), t("t5d"))

    nc.vector.reciprocal(out=rinv_dst, in_=t("ssum"))
    for j, nm in enumerate(["c0", "c1", "c2", "c3"]):
        eng = dve if j < 2 else po
        for kdup in range(2):
            eng.tensor_tensor(out=cj[j][:, :, kdup], in0=t(nm),
                              in1=rinv_dst, op=mybir.AluOpType.mult)


def _gate(nc, av, bv, cbc, u_t, v_t, ns):
    """h = (c3*b + c1)*a + (c2*b + c0), written into u_t[:, :ns].
    av/bv: [128, ns, 32] APs; cbc(j): coef broadcast [128, ns, 16, 2]."""
    av4 = av.rearrange("p n (a b) -> p n a b", b=2)
    bv4 = bv.rearrange("p n (a b) -> p n a b", b=2)
    u4 = u_t[:, :ns, :, :]
    v4 = v_t[:, :ns, :, :]
    nc.vector.tensor_mul(u4, bv4, cbc(3))
    nc.vector.tensor_add(u4, u4, cbc(1))
    nc.vector.tensor_mul(u4, u4, av4)
    nc.vector.tensor_mul(v4, bv4, cbc(2))
    nc.vector.tensor_add(v4, v4, cbc(0))
    nc.vector.tensor_add(u4, u4, v4)
    return u_t[:, :ns, :, :].rearrange("p n a b -> p n (a b)")


def build_nc(nbins=None):
    if nbins is None:
        nbins = _NC_CACHE["last_nbins"]
    _NC_CACHE["last_nbins"] = nbins
    key = ("nc", nbins)
    if key in _NC_CACHE:
        return _NC_CACHE[key]
    n2rows = nbins * 8
    nslot2 = n2rows // 128
    nc3 = nbins // 128          # block-columns for layer 3
    nslot3 = nc3 * 7            # coef slots for layer 3 (c, j)-flattened

    nc = bacc.Bacc("TRN2", target_bir_lowering=False, debug=False,
                   enable_asserts=False, num_devices=NCORES)

    a1_d = nc.dram_tensor("a1", [128, NSLOT, 32], BF16, kind="ExternalInput")
    b1_d = nc.dram_tensor("b1", [128, NSLOT, 32], BF16, kind="ExternalInput")
    nslots = [NSLOT, nslot2, nslot3]
    wf = [nc.dram_tensor(f"wf{l}", [128, 16, nslots[l]], BF16,
                         kind="ExternalInput") for l in range(3)]
    ia2 = nc.dram_tensor("ia2", [128, n2rows // 16], I16, kind="ExternalInput")
    ib2 = nc.dram_tensor("ib2", [128, n2rows // 16], I16, kind="ExternalInput")
    g10 = nc.dram_tensor("g10", [128, nc3, 7, K], BF16, kind="ExternalInput")
    h1_dram = nc.dram_tensor("h1", [W, E], BF16, kind="Internal")
    h2_dram = nc.dram_tensor("h2", [n2rows, 32], BF16, kind="Internal")
    out_dram = nc.dram_tensor("out", [K, BC], F32, kind="ExternalOutput")

    with tile.TileContext(nc) as tc:
        with (
            tc.tile_pool(name="persist", bufs=1) as persist,
            tc.tile_pool(name="coef", bufs=1) as coefp,
            tc.tile_pool(name="gath", bufs=2) as gath,
            tc.tile_pool(name="temps", bufs=2) as temps,
            tc.tile_pool(name="psum", bufs=1, space="PSUM") as psump,
        ):
            nc.gpsimd.load_library(mlp)

            g10_sb = persist.tile([128, nc3, 7, K], BF16, tag="g10")
            nc.sync.dma_start(g10_sb[:], g10[:])

            psum_out = psump.tile([K, BC], F32, tag="acc")
            n_mm = nc3 * 7
            mm_i = 0

            maxsl = max(nslots)

            def coef_prep(l, ctag):
                nsl = nslots[l]
                wf_t = coefp.tile([128, 16, maxsl], BF16, tag="wf",
                                  name=f"wf_sb{l}")
                wf_sb = wf_t[:, :, :nsl]
                nc.sync.dma_start(wf_sb, wf[l][:])
                e_t = coefp.tile([128, 16, maxsl], BF16, tag="e",
                                 name=f"e_sb{l}")
                e_sb = e_t[:, :, :nsl]
                nc.scalar.activation(e_sb, wf_sb,
                                     mybir.ActivationFunctionType.Exp)
                cj_t = [coefp.tile([128, maxsl, 2], BF16, tag=f"{ctag}{j}",
                                   name=f"cj{l}{j}") for j in range(4)]
                cj = [t[:, :nsl, :] for t in cj_t]
                rinv_t = coefp.tile([128, maxsl], F32, tag="rinv",
                                    name=f"rinv{l}")
                rinv = rinv_t[:, :nsl]

                def alloc(name, l=l):
                    return coefp.tile([128, maxsl], BF16, tag=f"ct_{name}",
                                      name=f"ct_{l}_{name}")
                _coef_tree(nc, alloc, e_sb, cj, rinv, nsl)
                return cj

            # ---- layer-3 chunk emitter (interleaved into layer-2 loop) ----
            l3_chunks = list(_chunks(nc3, CB3))
            l3_state = {"ptr": 0, "mm": 0, "cjv": None}

            def emit_l3_chunk(c0, ncb):
                cjv = l3_state["cjv"]
                hap = h2_dram.ap()
                t3 = gath.tile([128, CB3, 256], BF16, tag="l3")
                src = hap[c0 * 1024: (c0 + ncb) * 1024, :]
                src = src.rearrange("(c p r) e -> p c (r e)", p=128, r=8)
                nc.sync.dma_start(t3[:, :ncb, :], src)
                for j in range(7):
                    u_t = temps.tile([128, CB3, 16, 2], BF16, tag="u3")
                    v_t = temps.tile([128, CB3, 16, 2], BF16, tag="v3")

                    def cbc(q, c0=c0, ncb=ncb, j=j, cjv=cjv):
                        return (cjv[q][:, c0:c0 + ncb, j, :].unsqueeze(2)
                                .to_broadcast([128, ncb, 16, 2]))
                    uv = _gate(nc, t3[:, :ncb, 32 * j:32 * j + 32],
                               t3[:, :ncb, 32 * j + 32:32 * j + 64],
                               cbc, u_t, v_t, ncb)
                    for c in range(ncb):
                        nc.tensor.matmul(
                            psum_out[:],
                            lhsT=g10_sb[:, c0 + c, j, :],
                            rhs=uv[:, c, :],
                            start=(l3_state["mm"] == 0),
                            stop=(l3_state["mm"] == n_mm - 1),
                        )
                        l3_state["mm"] += 1

            def emit_ready_l3(slots_done):
                while l3_state["ptr"] < len(l3_chunks):
                    c0, ncb = l3_chunks[l3_state["ptr"]]
                    if (c0 + ncb) * 8 > slots_done:
                        break
                    emit_l3_chunk(c0, ncb)
                    l3_state["ptr"] += 1

            for l in range(2):
                cj = coef_prep(l, "c")
                if l == 1:
                    cj3 = coef_prep(2, "d")
                    l3_state["cjv"] = [
                        c.rearrange("p (c j) d -> p c j d", j=7) for c in cj3]

                if l == 0:
                    for s0, ns in _chunks(NSLOT):
                        a_t = gath.tile([128, CHUNK_SLOTS, 32], BF16, tag="a1c")
                        b_t = gath.tile([128, CHUNK_SLOTS, 32], BF16, tag="b1c")
                        nc.sync.dma_start(a_t[:, :ns, :], a1_d[:, s0:s0 + ns, :])
                        nc.sync.dma_start(b_t[:, :ns, :], b1_d[:, s0:s0 + ns, :])
                        u_t = temps.tile([128, CHUNK_SLOTS, 16, 2], BF16, tag="u")
                        v_t = temps.tile([128, CHUNK_SLOTS, 16, 2], BF16, tag="v")

                        def cbc(j, s0=s0, ns=ns, cj=cj):
                            return (cj[j][:, s0:s0 + ns, :].unsqueeze(2)
                                    .to_broadcast([128, ns, 16, 2]))
                        uv = _gate(nc, a_t[:, :ns, :], b_t[:, :ns, :], cbc,
                                   u_t, v_t, ns)
                        hap = h1_dram.ap()
                        dst = hap[s0 * 128: s0 * 128 + ns * 128, :32]
                        dst = dst.rearrange("(c p) e -> p c e", p=128)
                        nc.sync.dma_start(dst, uv)

                elif l == 1:
                    ia_sb = persist.tile([128, n2rows // 16], I16, tag="ia")
                    ib_sb = persist.tile([128, n2rows // 16], I16, tag="ib")
                    nc.sync.dma_start(ia_sb[:], ia2[:])
                    nc.sync.dma_start(ib_sb[:], ib2[:])
                    src_ap = h1_dram[H_BASE:W]
                    for s0, ns in _chunks(nslot2):
                        a_t = gath.tile([128, CHUNK_SLOTS, E], BF16, tag="a")
                        b_t = gath.tile([128, CHUNK_SLOTS, E], BF16, tag="b")
                        col = s0 * 8
                        slot = 0
                        for n in _gathers(ns):
                            ncols = n // 16
                            nsg = n // 128
                            nc.gpsimd.dma_gather(
                                a_t[:, slot:slot + nsg, :], src_ap,
                                ia_sb[:, col:col + ncols], n, n, E)
                            nc.gpsimd.dma_gather(
                                b_t[:, slot:slot + nsg, :], src_ap,
                                ib_sb[:, col:col + ncols], n, n, E)
                            col += ncols
                            slot += nsg
                        u_t = temps.tile([128, CHUNK_SLOTS, 16, 2], BF16, tag="u")
                        v_t = temps.tile([128, CHUNK_SLOTS, 16, 2], BF16, tag="v")

                        def cbc(j, s0=s0, ns=ns, cj=cj):
                            return (cj[j][:, s0:s0 + ns, :].unsqueeze(2)
                                    .to_broadcast([128, ns, 16, 2]))
                        uv = _gate(nc, a_t[:, :ns, :32], b_t[:, :ns, :32], cbc,
                                   u_t, v_t, ns)
                        hap = h2_dram.ap()
                        dst = hap[s0 * 128: s0 * 128 + ns * 128, :]
                        dst = dst.rearrange("(c p) e -> p c e", p=128)
                        nc.sync.dma_start(dst, uv)
                        emit_ready_l3(s0 + ns)

            emit_ready_l3(nslot2)
            assert l3_state["mm"] == n_mm and l3_state["ptr"] == len(l3_chunks)

            out_sb = persist.tile([K, BC], F32, tag="outsb")
            nc.scalar.mul(out_sb[:], psum_out[:], 1.0 / TAU)
            nc.sync.dma_start(out_dram[:], out_sb[:])

    nc.compile()
    _NC_CACHE[key] = nc
    return nc


def _wrap(idx):
    n = idx.shape[0]
    arr = np.empty((128, n // 16), dtype=np.int16)
    blk = idx.reshape(n // 16, 16).T.astype(np.int16)
    for g in range(8):
        arr[g * 16:(g + 1) * 16, :] = blk
    return arr


def _build_cover(a3, b3):
    """Path cover of the layer-3 access multigraph (vertices = layer-2
    logical neurons, edge k = (a3[k], b3[k])), chopped into <=8-row
    segments and bin-packed into 8-row bins.

    Returns (rows2, slack, slot_neuron, slot_flip, nbins): rows2[r] =
    layer-2 logical neuron at h2 row r (0 for slack rows), slot_neuron
    [bin, j] = layer-3 neuron whose inputs are rows (8*bin+j, 8*bin+j+1)
    (-1 = garbage slot), slot_flip = a/b orientation flip."""
    Edg = len(a3)
    head = np.full(W, -1, dtype=np.int64)
    nxt = np.empty(2 * Edg, dtype=np.int64)
    for k in range(Edg):
        u, v = a3[k], b3[k]
        nxt[2 * k] = head[u]
        head[u] = 2 * k          # half 2k stored at a-side (u -> v)
        nxt[2 * k + 1] = head[v]
        head[v] = 2 * k + 1      # half 2k+1 stored at b-side (v -> u)
    used = np.zeros(Edg, dtype=bool)
    deg = np.bincount(np.concatenate([a3, b3]), minlength=W)

    def walk(start):
        verts = [start]
        edges = []
        v = start
        while True:
            h = head[v]
            while h != -1 and used[h >> 1]:
                h = nxt[h]
            head[v] = h
            if h == -1:
                break
            k = int(h) >> 1
            used[k] = True
            if h & 1:            # stored at b-side: traversal b -> a
                w = int(a3[k])
                edges.append((k, True))
            else:
                w = int(b3[k])
                edges.append((k, False))
            verts.append(w)
            v = w
        return verts, edges

    paths = []
    for s in np.nonzero(deg % 2 == 1)[0]:
        verts, edges = walk(int(s))
        if edges:
            paths.append((verts, edges))
    for s in range(W):
        while True:
            verts, edges = walk(s)
            if not edges:
                break
            paths.append((verts, edges))
    assert used.all()

    # chop into segments of <= 8 rows (7 edges); cut vertex shared
    segs = []
    for verts, edges in paths:
        i = 0
        while i < len(edges):
            j = min(i + 7, len(edges))
            segs.append((verts[i:j + 1], edges[i:j]))
            i = j

    # bin-pack segments (2..8 rows) into 8-row bins
    from collections import defaultdict
    by_size = defaultdict(list)
    for sg in segs:
        by_size[len(sg[0])].append(sg)

    def pop_fit(cap):
        for s in range(cap, 1, -1):
            if by_size[s]:
                return by_size[s].pop()
        return None

    bins = []
    while True:
        sg = pop_fit(8)
        if sg is None:
            break
        cur = [sg]
        cap = 8 - len(sg[0])
        while cap >= 2:
            sg2 = pop_fit(cap)
            if sg2 is None:
                break
            cur.append(sg2)
            cap -= len(sg2[0])
        bins.append(cur)

    nbins = (len(bins) + 127) // 128 * 128
    while len(bins) < nbins:
        bins.append([])

    rows2 = np.zeros(nbins * 8, dtype=np.int64)
    slack = np.ones(nbins * 8, dtype=bool)
    slot_neuron = np.full((nbins, 7), -1, dtype=np.int64)
    slot_flip = np.zeros((nbins, 7), dtype=bool)
    for bi, bin_segs in enumerate(bins):
        r = 0
        for verts, edges in bin_segs:
            for q, v in enumerate(verts):
                rows2[bi * 8 + r + q] = v
                slack[bi * 8 + r + q] = False
            for q, (k, flip) in enumerate(edges):
                slot_neuron[bi, r + q] = k
                slot_flip[bi, r + q] = flip
            r += len(verts)
    return rows2, slack, slot_neuron, slot_flip, nbins


def kernel(x, w1, w2, w3, idx_a1, idx_b1, idx_a2, idx_b2, idx_a3, idx_b3):
    x = np.asarray(x, dtype=np.float32)
    ws = [np.asarray(w, dtype=np.float32) for w in (w1, w2, w3)]
    ias = [np.asarray(i).astype(np.int64) for i in (idx_a1, idx_a2, idx_a3)]
    ibs = [np.asarray(i).astype(np.int64) for i in (idx_b1, idx_b2, idx_b3)]

    rows2, slack, slot_neuron, slot_flip, nbins = _build_cover(ias[2], ibs[2])
    n2rows = nbins * 8
    nc3 = nbins // 128

    # layer-2 instance gather lists (h1 rows in logical order, rebased)
    ia2i = ias[1][rows2] - H_BASE
    ib2i = ibs[1][rows2] - H_BASE
    ia2i[slack] = 0
    ib2i[slack] = 0

    # trailing-idx fix: every 1024-idx gather sublist must end with idxs
    # >= 0 in both lists (SWDGE trims trailing negatives). The stream is
    # row order, so the sublist-final row is row 7 of bins 128k+127;
    # reorder bins to place "good" bins there.
    good = ((ia2i.reshape(-1, 8)[:, 7] >= 0)
            & (ib2i.reshape(-1, 8)[:, 7] >= 0))
    order = np.arange(nbins)
    bad_bound = [kk for kk in range(127, nbins, 128) if not good[kk]]
    spare = [int(g) for g in np.nonzero(good)[0] if (g % 128) != 127]
    assert len(spare) >= len(bad_bound), "not enough good bins"
    for bb, sp in zip(bad_bound, spare):
        order[bb], order[sp] = order[sp], order[bb]

    rows2 = rows2.reshape(nbins, 8)[order].reshape(-1)
    slack = slack.reshape(nbins, 8)[order].reshape(-1)
    ia2i = ia2i.reshape(nbins, 8)[order].reshape(-1)
    ib2i = ib2i.reshape(nbins, 8)[order].reshape(-1)
    slot_neuron = slot_neuron[order]
    slot_flip = slot_flip[order]
    assert (ia2i[1023::1024] >= 0).all() and (ib2i[1023::1024] >= 0).all()

    nc = build_nc(nbins)

    shared = {"ia2": _wrap(ia2i), "ib2": _wrap(ib2i)}
    # weights: layer 1 logical, layer 2 per-instance, layer 3 per-slot
    wf2 = ws[1][rows2]
    sn = slot_neuron.reshape(-1)
    sf = slot_flip.reshape(-1)
    valid = sn >= 0
    wf3 = np.zeros((nbins * 7, 16), dtype=np.float32)   # bin-major (bin*7+j)
    wf3[valid] = ws[2][sn[valid]]
    fl = valid & sf
    wf3[fl] = wf3[fl][:, SWP]
    for l, (wfl, nsl) in enumerate(
            [(ws[0], NSLOT), (wf2, n2rows // 128)]):
        shared[f"wf{l}"] = np.ascontiguousarray(
            wfl.reshape(nsl, 128, 16).transpose(1, 2, 0)
        ).astype(ml_dtypes.bfloat16)
    # layer-3 coef slots are (c, j)-flattened per partition: [p, g, c, j]
    shared["wf2"] = np.ascontiguousarray(
        wf3.reshape(nc3, 128, 7, 16).transpose(1, 3, 0, 2)
        .reshape(128, 16, nc3 * 7)
    ).astype(ml_dtypes.bfloat16)

    g10 = np.zeros((nbins * 7, K), dtype=np.float32)
    g10[valid, sn[valid] // (W // K)] = 1.0
    shared["g10"] = np.ascontiguousarray(
        g10.reshape(nc3, 128, 7, K).transpose(1, 0, 2, 3)
    ).astype(ml_dtypes.bfloat16)

    # host-gathered layer-1 inputs
    ga = x[:, ias[0]]
    gb = x[:, ibs[0]]
    in_maps = []
    for c in range(NCORES):
        sl = slice(c * BC, (c + 1) * BC)
        m = dict(shared)
        for nm, g in (("a1", ga), ("b1", gb)):
            arr = g[sl].T.reshape(NSLOT, 128, BC).transpose(1, 0, 2)
            m[nm] = np.ascontiguousarray(arr).astype(ml_dtypes.bfloat16)
        in_maps.append(m)

    res = run_bass_kernel_spmd(nc, in_maps, core_ids=list(range(NCORES)))

    out = np.empty((BATCH, K), dtype=np.float32)
    for c in range(NCORES):
        out[c * BC:(c + 1) * BC] = res.results[c]["out"].T
    return out
